# revision 59
# baseline (speedup 1.0000x reference)
"""Trainium2 Bass kernel for a pre-LN transformer block (B=4, T=2048, E=1024, H=16).

Sharding: 8 cores = 4 batches x 2 token-halves. Each core receives its batch's
full 2048 tokens (rolled so its own 1024 query tokens come first), computes
K/V for all 2048 tokens (redundantly with its pair core) and everything else
(Q, attention, proj, FFN) only for its own 1024 tokens. Zero cross-core
communication; host reassembles the output.

Structure: attention runs as two 512-query passes. Scores contract over the
full K=128 with zero-padded q (half-activity K=64 matmuls HAM-throttle the
PE to 1.2 GHz). Softmax exp is split across engines: part on ACT (table
exp -> fp8e4m3), part on DVE via a Schraudolph bit-trick
(rint(x*8/ln2 + 55.65) as int8, reinterpreted as fp8e4m3; max rel err ~7%,
which washes out over the softmax average and partially cancels against the
identically-approximated denominator). A@V runs in fp8 DoubleRow over
key-tile pairs; the stationary v operand's columns 64:127 are all ones, so
po rows 64:127 hold the softmax denominator replicated across 64 partitions
(one 64-lane reciprocal_approx_fast, no partition broadcast). Each pair's
normalize chain is deferred into the next pair's instruction stream to
avoid head-of-line blocking the in-order ACT/DVE queues. During pass 2 the
first half's attention projection, LN2 and FFN1 are interleaved into the
emission stream to overlap the exp-bound attention; FFN1 of half 1 is
interleaved with the first FFN2 wave. LN2 computes rstd with a quake-style
rsqrt on the DVE so the scalar engine's activation table never leaves the
exp set mid-pass. FFN stays bf16 (fp8 FFN exceeds the error budget).
"""

import numpy as np
import ml_dtypes

BF = ml_dtypes.bfloat16
F8 = ml_dtypes.float8_e4m3

B, T, E, H, HS, FF = 4, 2048, 1024, 16, 64, 4096
TQ = T // 2          # own query tokens per core
QW = 512             # query width per attention pass
NP = H // 2          # head pairs
NCORES = 8
EPS = 1e-5
NMT = T // 128       # 16 key tiles (full batch)
NST = NMT // 2       # 8 key-tile pairs (DoubleRow AV)
NMQ = TQ // 128      # 8 own token tiles
NJE = E // 128       # 8 feature tiles of E
NJF = FF // 128      # 32 feature tiles of FF

LN2_ = float(np.log(2.0))
SCH_A8 = (8.0 / LN2_) * 0.125      # fold HS^-0.5 score scale into schraudolph
SCH_B8 = 56.0 - 0.35
QUAKE_C = float(0x5F3759DF + 1)

_CACHE = {}
TRACE = False        # set by test harness to capture an NTFF profile
LAST_RESULTS = None  # BassKernelResults from the most recent run


def _build():
    import concourse.bacc as bacc
    import concourse.tile as tile
    from concourse import mybir
    from contextlib import ExitStack

    f32 = mybir.dt.float32
    bf16 = mybir.dt.bfloat16
    f8 = mybir.dt.float8e4
    i8 = mybir.dt.int8
    i32 = mybir.dt.int32
    DR = mybir.MatmulPerfMode.DoubleRow
    AF = mybir.ActivationFunctionType
    OP = mybir.AluOpType

    nc = bacc.Bacc("TRN2", target_bir_lowering=False, debug=False,
                   num_devices=NCORES)

    # ---- DRAM I/O ----
    x_d = nc.declare_dram_parameter("x", [T, E], bf16, isOutput=False)
    wq_d = nc.declare_dram_parameter("wq", [NJE, 128, E], f8, isOutput=False)
    wk_d = nc.declare_dram_parameter("wk", [NJE, 128, E], f8, isOutput=False)
    wv_d = nc.declare_dram_parameter("wv", [NJE, 128, E], f8, isOutput=False)
    wo_d = nc.declare_dram_parameter("wo", [NJE, 128, E], f8, isOutput=False)
    w1_d = nc.declare_dram_parameter("w1", [NJF, 128, E], bf16, isOutput=False)
    w2_d = nc.declare_dram_parameter("w2", [NJF, 128, E], bf16, isOutput=False)
    cq_d = nc.declare_dram_parameter("cq", [128, NJE], f32, isOutput=False)
    ck_d = nc.declare_dram_parameter("ck", [128, NJE], f32, isOutput=False)
    cvb_d = nc.declare_dram_parameter("cvb", [128, E], f32, isOutput=False)
    xq_d = nc.declare_dram_parameter("xq", [TQ, E], bf16, isOutput=False)
    b2b_d = nc.declare_dram_parameter("b2b", [128, E], f32, isOutput=False)
    b1c_d = nc.declare_dram_parameter("b1c", [128, NJF], f32, isOutput=False)
    out_d = nc.declare_dram_parameter("out", [TQ, E], f32, isOutput=True)

    def layernorm1(stats_pool, x_sb, out_bf, eps_sb):
        # LN with ACT sqrt (runs before any exp -> no act-table thrash)
        st = stats_pool.tile([128, 2, 6], f32, name="ln_st")
        nc.vector.bn_stats(out=st[:, 0, :], in_=x_sb[:, 0:512])
        nc.vector.bn_stats(out=st[:, 1, :], in_=x_sb[:, 512:1024])
        mv = stats_pool.tile([128, 2], f32, name="ln_mv")
        nc.vector.bn_aggr(out=mv[:], in_=st[:])
        rstd = stats_pool.tile([128, 1], f32, name="ln_rstd")
        nc.scalar.activation(out=rstd[:], in_=mv[:, 1:2], func=AF.Sqrt,
                             bias=eps_sb[:])
        nc.vector.reciprocal(out=rstd[:], in_=rstd[:])
        nmr = stats_pool.tile([128, 1], f32, name="ln_nmr")
        nc.vector.tensor_tensor(out=nmr[:], in0=mv[:, 0:1], in1=rstd[:],
                                op=OP.mult)
        nc.vector.tensor_scalar_mul(out=nmr[:], in0=nmr[:], scalar1=-1.0)
        nc.scalar.activation(out=out_bf[:], in_=x_sb[:], func=AF.Identity,
                             bias=nmr[:], scale=rstd[:])

    def layernorm2(stats_pool, x_sb, out_bf):
        # LN with quake rsqrt on DVE (no ACT Sqrt -> exp table set stays hot)
        st = stats_pool.tile([128, 2, 6], f32, name="ln_st")
        nc.vector.bn_stats(out=st[:, 0, :], in_=x_sb[:, 0:512])
        nc.vector.bn_stats(out=st[:, 1, :], in_=x_sb[:, 512:1024])
        mv = stats_pool.tile([128, 2], f32, name="ln_mv")
        nc.vector.bn_aggr(out=mv[:], in_=st[:])
        t = stats_pool.tile([128, 1], f32, name="ln_t")
        nc.vector.tensor_scalar_add(out=t[:], in0=mv[:, 1:2], scalar1=EPS)
        a = stats_pool.tile([128, 1], i32, name="ln_a")
        nc.vector.tensor_scalar(out=a[:], in0=t[:].bitcast(i32),
                                scalar1=1, scalar2=None,
                                op0=OP.arith_shift_right)
        nc.vector.tensor_tensor(out=a[:], in0=a[:], in1=a[:], op=OP.bitwise_not)
        nc.vector.tensor_scalar(out=a[:], in0=a[:], scalar1=QUAKE_C,
                                scalar2=None, op0=OP.add)
        y0 = a[:].bitcast(f32)
        w = stats_pool.tile([128, 1], f32, name="ln_w")
        nc.vector.tensor_tensor(out=w[:], in0=y0, in1=y0, op=OP.mult)
        nc.vector.tensor_tensor(out=w[:], in0=w[:], in1=t[:], op=OP.mult)
        nc.vector.tensor_scalar(out=w[:], in0=w[:], scalar1=-0.5, scalar2=1.5,
                                op0=OP.mult, op1=OP.add)
        rstd = stats_pool.tile([128, 1], f32, name="ln_rstd")
        nc.vector.tensor_tensor(out=rstd[:], in0=w[:], in1=y0, op=OP.mult)
        nmr = stats_pool.tile([128, 1], f32, name="ln_nmr")
        nc.vector.tensor_tensor(out=nmr[:], in0=mv[:, 0:1], in1=rstd[:],
                                op=OP.mult)
        nc.vector.tensor_scalar_mul(out=nmr[:], in0=nmr[:], scalar1=-1.0)
        nc.scalar.activation(out=out_bf[:], in_=x_sb[:], func=AF.Identity,
                             bias=nmr[:], scale=rstd[:])

    with tile.TileContext(nc) as tc:
        top = ExitStack()
        const = top.enter_context(tc.tile_pool(name="const", bufs=1, side="left"))
        eps_sb = const.tile([128, 1], f32)
        nc.vector.memset(eps_sb[:], EPS)
        cq_sb = const.tile([128, NJE], f32)
        nc.sync.dma_start(out=cq_sb[:], in_=cq_d[:])
        ck_sb = const.tile([128, NJE], f32)
        nc.sync.dma_start(out=ck_sb[:], in_=ck_d[:])
        b1_sb = const.tile([128, NJF], f32)
        nc.gpsimd.dma_start(out=b1_sb[:], in_=b1c_d[:])

        # xq pool pushed first on the right stack: it outlives qkvact
        # (used by proj(q1) during FFN2 wave A)
        pxp_es = ExitStack()
        pxp = pxp_es.enter_context(tc.tile_pool(name="proj_x", bufs=2, side="right"))
        # pools for QKV created before LN1 so the weight DMAs can be
        # interleaved with the x loads on the sync queue (the gpsimd queue
        # reorders memsets ahead of DMA issues, starving the first matmuls)
        qkv_es = ExitStack()
        qkv_pool = qkv_es.enter_context(tc.tile_pool(name="qkvact", bufs=1, side="right"))
        # q^T zero-padded per head (head h in rows (h%2)*64, zeros elsewhere)
        # so scores contract over K=128: half-activity K=64 matmuls would HAM-
        # throttle the PE to 1.2 GHz (measured: whole attention region slows)
        qT = qkv_pool.tile([128, H, TQ], bf16)
        kT = qkv_pool.tile([128, NJE, T], bf16)       # k^T (all tokens)
        # v in cols 0:64; cols 64:128 all ones so A@V replicates the softmax
        # denominator onto po rows 64:127 (64-partition reciprocal + no
        # broadcast needed) while keeping the PE at full M=128 activity
        v8 = qkv_pool.tile([128, NST, 2, H, 128], f8)
        wqkv_es = ExitStack()
        wqkv = wqkv_es.enter_context(tc.tile_pool(name="w_pool", bufs=2, side="right"))
        cv_sb = wqkv.tile([128, E], bf16, name="cv")
        wk_sb = wqkv.tile([128, NJE, E], f8, name="wt")
        wq_sb = wqkv.tile([128, NJE, E], f8, name="wt")
        wv_sb = wqkv.tile([128, NJE, E], f8, name="wt")

        # ---------- Phase 1: LN1 + transpose (+ QKV weight loads) ----------
        hT_es = ExitStack()
        hT_pool = hT_es.enter_context(tc.tile_pool(name="hT", bufs=1, side="left"))
        hG = [hT_pool.tile([128, 4, NJE, 128], bf16, name=f"hG{g}")
              for g in range(4)]
        h8 = [hT_pool.tile([128, NJE, 4, 128], f8, name=f"h8{g}")
              for g in range(4)]
        with tc.tile_pool(name="ln1", bufs=16, side="left") as xin, \
             tc.tile_pool(name="ln1s", bufs=10, side="left") as stp, \
             tc.tile_pool(name="ln1h", bufs=3, side="left") as hbp:
            xt = []
            for mt in range(NMT):
                x_sb = xin.tile([128, E], bf16)
                nc.sync.dma_start(out=x_sb[:], in_=x_d[mt * 128:(mt + 1) * 128, :])
                xt.append(x_sb)
                if mt == 3:
                    for j in range(NJE):
                        nc.sync.dma_start(out=wk_sb[:, j, :], in_=wk_d[j])
                elif mt == 7:
                    for j in range(NJE):
                        nc.sync.dma_start(out=wq_sb[:, j, :], in_=wq_d[j])

            def cast_h8(mt):
                nc.vector.tensor_copy(out=h8[mt // 4][:, :, mt % 4, :],
                                      in_=hG[mt // 4][:, mt % 4, :, :])
            for mt in range(NMT):
                h_bf = hbp.tile([128, E], bf16)
                layernorm1(stp, xt[mt], h_bf, eps_sb)
                nc.sync.dma_start_transpose(out=hG[mt // 4][:, mt % 4, :, :],
                                            in_=h_bf[:])
                # cast deferred 2 tiles so the DVE never head-of-line-waits
                # on the transpose DMA
                if mt >= 2:
                    cast_h8(mt - 2)
            cast_h8(NMT - 2)
            cast_h8(NMT - 1)

        # ---------- Phase 2: QKV projections ----------
        with tc.tile_pool(name="qkv_ps", bufs=8, space="PSUM") as qkps:
            for j in range(NJE):
                nc.gpsimd.dma_start(out=wv_sb[:, j, :], in_=wv_d[j])
            nc.gpsimd.dma_start(out=cv_sb[:], in_=cvb_d[:])
            nc.gpsimd.memset(v8[:, :, :, :, 64:128], 1.0)
            # k for all 2048 tokens
            for g in range(4):
                for mf in range(NJE):
                    pk = qkps.tile([128, 512], f32, name="ps_qkv")
                    for j in range(0, NJE, 2):
                        lhsT = wk_sb[:, j:j + 2, mf * 128:(mf + 1) * 128]
                        rhs = h8[g][:, j:j + 2, :, :]
                        nc.tensor.matmul(pk[:], lhsT, rhs, perf_mode=DR,
                                         start=(j == 0), stop=(j == NJE - 2))
                    nc.scalar.activation(out=kT[:, mf, g * 512:(g + 1) * 512],
                                         in_=pk[:], func=AF.Identity,
                                         bias=ck_sb[:, mf:mf + 1])
            # q for own 1024 tokens (g=0 -> first pass half, g=1 -> second)
            for g in range(2):
                for mf in range(NJE):
                    pq = qkps.tile([128, 512], f32, name="ps_qkv")
                    for j in range(0, NJE, 2):
                        lhsT = wq_sb[:, j:j + 2, mf * 128:(mf + 1) * 128]
                        rhs = h8[g][:, j:j + 2, :, :]
                        nc.tensor.matmul(pq[:], lhsT, rhs, perf_mode=DR,
                                         start=(j == 0), stop=(j == NJE - 2))
                    sl = slice(g * 512, (g + 1) * 512)
                    nc.scalar.activation(out=qT[0:64, 2 * mf, sl], in_=pq[0:64, :],
                                         func=AF.Identity,
                                         bias=cq_sb[0:64, mf:mf + 1])
                    nc.scalar.activation(out=qT[64:128, 2 * mf + 1, sl],
                                         in_=pq[64:128, :],
                                         func=AF.Identity,
                                         bias=cq_sb[64:128, mf:mf + 1])
            for h in range(H):
                p0 = 64 - (h % 2) * 64  # zero the OTHER head's rows
                nc.gpsimd.memset(qT[p0:p0 + 64, h, :], 0.0)
            # v for all 2048 tokens
            for st in range(NMT):
                pv0 = qkps.tile([128, 512], f32, name="ps_qkv")
                pv1 = qkps.tile([128, 512], f32, name="ps_qkv")
                for j in range(0, NJE, 2):
                    lhsT = h8[st // 4][:, j:j + 2, st % 4, :]
                    nc.tensor.matmul(pv0[:], lhsT,
                                     wv_sb[:, j:j + 2, 0:512], perf_mode=DR,
                                     start=(j == 0), stop=(j == NJE - 2))
                    nc.tensor.matmul(pv1[:], lhsT,
                                     wv_sb[:, j:j + 2, 512:1024], perf_mode=DR,
                                     start=(j == 0), stop=(j == NJE - 2))
                nc.vector.tensor_tensor(
                    out=v8[:, st // 2, st % 2, 0:8, 0:HS],
                    in0=pv0.rearrange("p (h d) -> p h d", h=8),
                    in1=cv_sb[:, 0:512].rearrange("p (h d) -> p h d", h=8),
                    op=OP.add)
                nc.vector.tensor_tensor(
                    out=v8[:, st // 2, st % 2, 8:16, 0:HS],
                    in0=pv1.rearrange("p (h d) -> p h d", h=8),
                    in1=cv_sb[:, 512:1024].rearrange("p (h d) -> p h d", h=8),
                    op=OP.add)

        wqkv_es.close()
        hT_es.close()

        # ---------- persistent activations for attention + FFN ----------
        oT = top.enter_context(tc.tile_pool(name="oT", bufs=1, side="left")) \
            .tile([128, NJE, TQ], f8)                  # normalized attn out^T
        wop = top.enter_context(tc.tile_pool(name="proj_w", bufs=1, side="left"))
        wo_sb = wop.tile([128, NJE, E], f8)
        for j in range(NJE):
            nc.sync.dma_start(out=wo_sb[:, j, :], in_=wo_d[j])
        xr_pool = top.enter_context(tc.tile_pool(name="xr", bufs=1, side="left"))
        xr_t = [xr_pool.tile([128, E], bf16, name=f"xr{i}") for i in range(NMQ)]
        ffnT_es = ExitStack()
        ffnT0 = ffnT_es.enter_context(tc.tile_pool(name="ffnT0", bufs=1, side="left")) \
            .tile([128, NJF, QW], bf16)

        # ---------- attention pass machinery ----------
        def mk_finish(p, h, po2, qsl, rp, rbp, den_act):
            # po2[:, h%2] rows 64:127 hold the denominator (all-ones v8 cols).
            # Three ops, returned as closures so the caller can interleave
            # them into the NEXT pair's instruction stream: emitted
            # back-to-back they head-of-line-block the in-order ACT/DVE
            # queues at every pair boundary.
            ctx = {}
            def op1():
                ctx['den'] = rp.tile([64, QW], f32, name="rsum")
                if den_act or h % 2 == 0:
                    nc.scalar.copy(out=ctx['den'][:], in_=po2[64:128, h % 2, :])
                else:
                    nc.vector.tensor_copy(out=ctx['den'][:],
                                          in_=po2[64:128, h % 2, :])
            def op2():
                ctx['rb'] = rbp.tile([64, QW], f32, name="rbc")
                nc.vector.reciprocal_approx_fast(out=ctx['rb'][:], in_=ctx['den'][:])
            def op3():
                p0 = (h % 2) * 64
                nc.vector.tensor_tensor(out=oT[p0:p0 + 64, p, qsl],
                                        in0=po2[0:HS, h % 2, :], in1=ctx['rb'][:],
                                        op=OP.mult)
            return [op1, op2, op3]

        def emit_pass(qh, aps, ops, atp, rp, rbp, fillers, defer, den_act):
            qsl = slice(qh * QW, (qh + 1) * QW)
            finq = []
            for p in range(NP):
                # both heads' AV accumulators share one 2-bank tile
                po2 = ops.tile([128, 2, QW], f32, name="po2")
                ate = ato = None
                pendq = []
                for st in range(NMT):
                    psA = aps.tile([128, QW], f32, name="ps_sc")
                    psB = aps.tile([128, QW], f32, name="ps_sc")
                    ksl = slice(st * 128, (st + 1) * 128)
                    nc.tensor.matmul(psA[:], kT[:, p, ksl], qT[:, 2 * p, qsl],
                                     start=True, stop=True)
                    nc.tensor.matmul(psB[:], kT[:, p, ksl], qT[:, 2 * p + 1, qsl],
                                     start=True, stop=True)
                    if len(pendq) >= 2:
                        emit_av(p, *pendq.pop(0))
                    if st % 2 == 0:
                        ate = atp.tile([128, 2, QW], f8, name="at_e")
                        ato = atp.tile([128, 2, QW], i8, name="at_o")
                    nc.scalar.activation(out=ate[:, st % 2, :], in_=psA[:],
                                         func=AF.Exp, scale=float(HS) ** -0.5)
                    if st % 8 == 7:  # shift ~6% of odd-head exps to ACT
                        nc.scalar.activation(out=ato[:, st % 2, :].bitcast(f8),
                                             in_=psB[:], func=AF.Exp,
                                             scale=float(HS) ** -0.5)
                    else:
                        nc.vector.tensor_scalar(out=ato[:, st % 2, :], in0=psB[:],
                                                scalar1=SCH_A8, scalar2=SCH_B8,
                                                op0=OP.mult, op1=OP.add)
                    if st % 2 == 1:
                        pendq.append((st // 2, ate, ato, po2))
                    if finq:
                        finq.pop(0)()
                while pendq:
                    emit_av(p, *pendq.pop(0))
                fin = mk_finish(p, 2 * p, po2, qsl, rp, rbp, den_act) + \
                    mk_finish(p, 2 * p + 1, po2, qsl, rp, rbp, den_act)
                if defer and p < NP - 1:
                    finq = fin
                else:
                    for f in fin:
                        f()
                if fillers:
                    fillers.pop(0)()

        def emit_av(p, stp, ate, ato, po2):
            nc.tensor.matmul(po2[:, 0, :], v8[:, stp, :, 2 * p, :], ate[:],
                             perf_mode=DR, start=(stp == 0), stop=(stp == NST - 1))
            nc.tensor.matmul(po2[:, 1, :], v8[:, stp, :, 2 * p + 1, :],
                             ato[:].bitcast(f8),
                             perf_mode=DR, start=(stp == 0), stop=(stp == NST - 1))

        def emit_proj_mt(mt, pfp, stp2, hbp2, h2T):
            # attn projection + residual + LN2 + transpose for one token tile
            x_sb = pxp.tile([128, E], bf16, name="xq")
            nc.sync.dma_start(out=x_sb[:], in_=xq_d[mt * 128:(mt + 1) * 128, :])
            pa = pfp.tile([128, 512], f32, name="ps_pf")
            pb = pfp.tile([128, 512], f32, name="ps_pf")
            for j in range(0, NJE, 2):
                lhsT = oT[:, j:j + 2, mt * 128:(mt + 1) * 128]
                nc.tensor.matmul(pa[:], lhsT,
                                 wo_sb[:, j:j + 2, 0:512], perf_mode=DR,
                                 start=(j == 0), stop=(j == NJE - 2))
                nc.tensor.matmul(pb[:], lhsT,
                                 wo_sb[:, j:j + 2, 512:1024], perf_mode=DR,
                                 start=(j == 0), stop=(j == NJE - 2))
            nc.vector.tensor_tensor(out=xr_t[mt][:, 0:512], in0=pa[:],
                                    in1=x_sb[:, 0:512], op=OP.add)
            nc.vector.tensor_tensor(out=xr_t[mt][:, 512:1024], in0=pb[:],
                                    in1=x_sb[:, 512:1024], op=OP.add)
            h_bf = hbp2.tile([128, E], bf16)
            layernorm2(stp2, xr_t[mt][:], h_bf)
            nc.sync.dma_start_transpose(out=h2T[:, mt % 4, :, :], in_=h_bf[:])

        def emit_ffn1_mf(mf, qh, pfp, f1wp, h2T, ffnT):
            w1_sb = f1wp.tile([128, NJE, 128], bf16, name="w1t")
            nc.gpsimd.dma_start(out=w1_sb[:],
                                in_=w1_d[mf].rearrange("p (j c) -> p j c", j=NJE))
            pf = pfp.tile([128, 512], f32, name="ps_pf")
            for j in range(NJE):
                nc.tensor.matmul(pf[:], w1_sb[:, j, :], h2T[:, :, j, :],
                                 start=(j == 0), stop=(j == NJE - 1))
            nc.scalar.activation(out=ffnT[:, mf, :], in_=pf[:], func=AF.Relu,
                                 bias=b1_sb[:, mf:mf + 1])

        # ---------- Phase 3: attention pass 1 (queries 0:512) ----------
        att_es = ExitStack()
        atp = att_es.enter_context(tc.tile_pool(name="att_t", bufs=6, side="right"))
        rp = att_es.enter_context(tc.tile_pool(name="att_r", bufs=1, side="right"))
        rbp = att_es.enter_context(tc.tile_pool(name="att_rb", bufs=2, side="right"))
        with tc.tile_pool(name="att_ps1", bufs=6, space="PSUM") as aps1, \
             tc.tile_pool(name="att_po1", bufs=1, space="PSUM") as ops1:
            emit_pass(0, aps1, ops1, atp, rp, rbp, [], defer=True, den_act=True)

        # ---------- Phase 4: pass 2 + interleaved proj/LN2/FFN1 of half 0 ----------
        ln2_es = ExitStack()
        stp2 = ln2_es.enter_context(tc.tile_pool(name="ln2s", bufs=10, side="left"))
        hbp2 = ln2_es.enter_context(tc.tile_pool(name="ln2h", bufs=3, side="left"))
        h2T0_es = ExitStack()
        h2T0 = h2T0_es.enter_context(tc.tile_pool(name="h2T0", bufs=1, side="left")) \
            .tile([128, 4, NJE, 128], bf16)
        f1w_es = ExitStack()
        f1wp = f1w_es.enter_context(tc.tile_pool(name="f1w", bufs=4, side="left"))
        with tc.tile_pool(name="att_ps2", bufs=4, space="PSUM") as aps2, \
             tc.tile_pool(name="att_po2", bufs=1, space="PSUM") as ops2, \
             tc.tile_pool(name="pf_ps", bufs=2, space="PSUM") as pfp:
            fillers = []
            for mt in range(4):
                fillers.append(lambda mt=mt: emit_proj_mt(mt, pfp, stp2, hbp2, h2T0))
            for c in range(4):
                def f(c=c):
                    for mf in range(c * 8, (c + 1) * 8):
                        emit_ffn1_mf(mf, 0, pfp, f1wp, h2T0, ffnT0)
                fillers.append(f)
            emit_pass(1, aps2, ops2, atp, rp, rbp, fillers, defer=True, den_act=True)
        h2T1 = h2T0  # reuse the same region: half 0's transposes are consumed
        att_es.close()
        qkv_es.close()

        ffnT1_es = ExitStack()
        ffnT1 = ffnT1_es.enter_context(tc.tile_pool(name="ffnT1", bufs=1, side="right")) \
            .tile([128, NJF, QW], bf16)
        f2_es = ExitStack()
        f2wp = f2_es.enter_context(tc.tile_pool(name="f2w", bufs=8, side="right"))
        f2cp = f2_es.enter_context(tc.tile_pool(name="f2c", bufs=1, side="right"))
        f2op = f2_es.enter_context(tc.tile_pool(name="f2o", bufs=3, side="right"))
        b2_sb = f2cp.tile([128, E], f32)
        nc.gpsimd.dma_start(out=b2_sb[:], in_=b2b_d[:])

        def evict_f2(psum, mt, nbh):
            o_sb = f2op.tile([128, 512], f32, name="osb")
            nc.vector.tensor_tensor(out=o_sb[:], in0=psum[:],
                                    in1=xr_t[mt][:, nbh * 512:(nbh + 1) * 512],
                                    op=OP.add)
            nc.vector.tensor_tensor(out=o_sb[:], in0=o_sb[:],
                                    in1=b2_sb[:, nbh * 512:(nbh + 1) * 512],
                                    op=OP.add)
            nc.sync.dma_start(
                out=out_d[mt * 128:(mt + 1) * 128, nbh * 512:(nbh + 1) * 512],
                in_=o_sb[:])

        # FFN1 of half 1 interleaved 1:1 with FFN2 wave A (mt0-3 x nbh0,
        # reads only ffnT0) so the PE stays busy while the LN2(q1) chain
        # completes
        with tc.tile_pool(name="pf_ps2", bufs=3, space="PSUM") as pfp2, \
             tc.tile_pool(name="f2psA", bufs=4, space="PSUM") as f2psA:
            psA_ = [f2psA.tile([128, 512], f32, name="ps_f2a") for _ in range(4)]
            for k in range(NJF):
                w2_sb = f2wp.tile([128, 512], bf16, name="w2a")
                nc.gpsimd.dma_start(out=w2_sb[:], in_=w2_d[k][:, 0:512])
                # FFN2-A first: it depends only on ffnT0, so it fills the PE
                # right at pass-2 end; proj(q1) rides along on pfp2; FFN1
                # staggered 8 k behind (its h2T1 inputs come from proj q1)
                for mt in range(4):
                    nc.tensor.matmul(psA_[mt][:],
                                     ffnT0[:, k, mt * 128:(mt + 1) * 128],
                                     w2_sb[:],
                                     start=(k == 0), stop=(k == NJF - 1))
                if k % 2 == 0 and k < 8:
                    emit_proj_mt(4 + k // 2, pfp2, stp2, hbp2, h2T1)
                if k >= 8:
                    emit_ffn1_mf(k - 8, 1, pfp2, f1wp, h2T1, ffnT1)
            for mf in range(NJF - 8, NJF):
                emit_ffn1_mf(mf, 1, pfp2, f1wp, h2T1, ffnT1)
            for mt in range(4):
                evict_f2(psA_[mt], mt, 0)
        f1w_es.close()
        h2T0_es.close()
        ln2_es.close()

        # FFN2 waves B: [mt0-3, nbh1] + [mt4-7, nbh0], then C: [mt4-7, nbh1]
        with tc.tile_pool(name="f2ps", bufs=8, space="PSUM") as f2ps:
            psums = [f2ps.tile([128, 512], f32, name="ps_f2") for _ in range(8)]
            for k in range(NJF):
                w2_sb = f2wp.tile([128, E], bf16, name="w2f")
                nc.gpsimd.dma_start(out=w2_sb[:], in_=w2_d[k])
                for i in range(4):
                    nc.tensor.matmul(psums[i][:],
                                     ffnT0[:, k, i * 128:(i + 1) * 128],
                                     w2_sb[:, 512:1024],
                                     start=(k == 0), stop=(k == NJF - 1))
                    nc.tensor.matmul(psums[4 + i][:],
                                     ffnT1[:, k, i * 128:(i + 1) * 128],
                                     w2_sb[:, 0:512],
                                     start=(k == 0), stop=(k == NJF - 1))
            for i in range(4):
                evict_f2(psums[i], i, 1)
            for i in range(4):
                evict_f2(psums[4 + i], 4 + i, 0)
            psumsD = [f2ps.tile([128, 512], f32, name="ps_f2") for _ in range(4)]
            for k in range(NJF):
                w2_sb = f2wp.tile([128, 512], bf16, name="w2a")
                nc.gpsimd.dma_start(out=w2_sb[:], in_=w2_d[k][:, 512:1024])
                for i in range(4):
                    nc.tensor.matmul(psumsD[i][:],
                                     ffnT1[:, k, i * 128:(i + 1) * 128],
                                     w2_sb[:],
                                     start=(k == 0), stop=(k == NJF - 1))
            for i in range(4):
                evict_f2(psumsD[i], 4 + i, 1)
        f2_es.close()
        ffnT1_es.close()
        pxp_es.close()
        ffnT_es.close()

        top.close()

    nc.compile()
    return nc


def _prep_weights(ln1_g, ln1_b, Wq, Wk, Wv, Wo, bo, ln2_g, ln2_b, W1, b1, W2, b2):
    f64 = np.float64
    g1 = np.asarray(ln1_g, f64)
    b1ln = np.asarray(ln1_b, f64)
    g2 = np.asarray(ln2_g, f64)
    b2ln = np.asarray(ln2_b, f64)

    def flat_qkv(W):
        return np.asarray(W, f64).transpose(1, 0, 2).reshape(E, H * HS)

    Wqf, Wkf, Wvf = flat_qkv(Wq), flat_qkv(Wk), flat_qkv(Wv)
    out = {}
    out["wq"] = np.ascontiguousarray((g1[:, None] * Wqf).reshape(NJE, 128, E).astype(F8))
    out["wk"] = np.ascontiguousarray((g1[:, None] * Wkf).reshape(NJE, 128, E).astype(F8))
    out["wv"] = np.ascontiguousarray((g1[:, None] * Wvf).reshape(NJE, 128, E).astype(F8))
    cq = (b1ln @ Wqf).astype(np.float32)
    ck = (b1ln @ Wkf).astype(np.float32)
    cv = (b1ln @ Wvf).astype(np.float32)
    out["cq"] = np.ascontiguousarray(cq.reshape(NJE, 128).T)
    out["ck"] = np.ascontiguousarray(ck.reshape(NJE, 128).T)
    out["cvb"] = np.ascontiguousarray(np.broadcast_to(cv, (128, E)))
    out["wo"] = np.ascontiguousarray(np.asarray(Wo, f64).reshape(NJE, 128, E).astype(F8))
    W1p = g2[:, None] * np.asarray(W1, f64)
    b1p = (np.asarray(b1, f64) + b2ln @ np.asarray(W1, f64)).astype(np.float32)
    out["w1"] = np.ascontiguousarray(
        W1p.reshape(NJE, 128, NJF, 128).transpose(2, 1, 0, 3).reshape(NJF, 128, E).astype(BF))
    out["b1c"] = np.ascontiguousarray(b1p.reshape(NJF, 128).T)
    out["w2"] = np.ascontiguousarray(np.asarray(W2, f64).reshape(NJF, 128, E).astype(BF))
    out["b2b"] = np.ascontiguousarray(
        np.broadcast_to(np.asarray(b2, np.float32), (128, E)))
    return out


def kernel(x, ln1_g, ln1_b, Wq, Wk, Wv, Wo, bo, ln2_g, ln2_b, W1, b1, W2, b2):
    global LAST_RESULTS
    from concourse.bass_utils import run_bass_kernel_spmd

    if "nc" not in _CACHE:
        _CACHE["nc"] = _build()
    nc = _CACHE["nc"]

    wmap = _prep_weights(ln1_g, ln1_b, Wq, Wk, Wv, Wo, bo,
                         ln2_g, ln2_b, W1, b1, W2, b2)
    x = np.asarray(x, np.float32)

    in_maps = []
    for c in range(NCORES):
        b, half = c // 2, c % 2
        xb = x[b]
        x_roll = np.ascontiguousarray(
            np.concatenate([xb[half * TQ:], xb[:half * TQ]], axis=0))
        m = dict(wmap)
        m["x"] = x_roll.astype(BF)
        m["xq"] = np.ascontiguousarray(
            (x_roll[:TQ] + np.asarray(bo, np.float32)[None, :]).astype(BF))
        in_maps.append(m)

    res = run_bass_kernel_spmd(nc, in_maps, list(range(NCORES)), trace=TRACE)
    LAST_RESULTS = res

    out = np.empty((B, T, E), np.float32)
    for c in range(NCORES):
        b, half = c // 2, c % 2
        out[b, half * TQ:(half + 1) * TQ] = res.results[c]["out"]
    return out
